# revision 19
# baseline (speedup 1.0000x reference)
"""AttentionFusion kernel for Trainium2, 8 NeuronCores, data-parallel over B*S.

Math (per token t of B*S=8192, per head h of 16):
  q_i = x_i @ Wq (etc.), scores[i,j] = q_i . k_j / 8, w = softmax_j
  fused = mean_i(w @ v) @ Wo + x_0 ; out = LayerNorm(fused)
Mean-pool identity: mean_i att_i = sum_j wbar_j * v_j with
wbar_j = mean_i softmax(scores)[i, j].

v2 design:
- Host ships x pre-transposed (c-major) and pre-quantized: xh = fp8(x),
  xl = fp8((x-xh)*16), plus x0 (adapter 0, token-major f16) for the residual.
- Q,K: feature-major [f, t] via fp8 DoubleRow matmuls (plain fp8 - softmax
  attenuates the quantization noise).
- V: token-major [t, f] via 3-term error-compensated fp8 DoubleRow
  (xh@Wvh + (xl@Wvh + xh@Wvl)/16), combined from two PSUM groups on DVE.
- Scores: z = qT*kT elementwise (DVE, bf16), then per-head segment sums via
  PE indicator matmuls (contraction over feature partitions) -> s[16h, i, j, t].
- exp fused into the PSUM->SBUF copy (ACT), tiny PE transposes back to
  token-major, softmax + wbar on DVE at full 128-partition occupancy.
- wbar expanded head->features on ACT; weighted v-sum on DVE (bf16).
- O-projection in bf16 (accuracy), LayerNorm tail with bf16 fused and a
  single 4x-mode tensor_scalar normalize; f16 output.
"""

import sys

sys.path.insert(0, "/opt/trn_rl_repo")

import numpy as np

A, B, S, H = 4, 4, 2048, 1024
NH, HD = 16, 64
NCORES = 8
T = (B * S) // NCORES  # 1024 tokens per core
PB = 128
TB = T // PB  # 8 token blocks per core
KB = H // PB  # 8 contraction blocks
KP = KB // 2  # 4 DoubleRow contraction pairs
FB = H // PB  # 8 feature blocks
EPS = 1e-5

_BUILD_CACHE = {}


def _patch_tile_drain():
    """This walrus build allows only ONE sem-wait per CTRL instruction; the
    Tile kernel-tail drain can carry several. Split them onto NOPs."""
    import concourse.tile as tile
    from concourse import mybir
    from bass_rust import ScopedClock

    def _drain_and_barrier(self, tick_clock, wait_clock):
        nc = self.nc
        pre = nc.sync.nop(nofuse=True)
        wait_clock.add_sem_waits(pre.ins, ScopedClock({None: tick_clock.global_clock}))
        si = pre.ins.sync_info
        if si is not None and len(si.on_wait) > 1:
            waits = list(si.on_wait)
            si.on_wait = waits[:1]
            for i in range(1, len(waits)):
                nop = nc.sync.nop(nofuse=True)
                nop.ins.sync_info = mybir.SyncInfo(on_wait=waits[i : i + 1], on_update=[])
        nc.sync.drain()
        nc.all_engine_barrier()
        assert self.sems is not None
        popped = nc._tile_sem_poison_stack.pop()
        assert popped is self._sem_poison
        nc.clear_and_free_semaphores(list(self.sems.allocated().values()))
        nc.all_engine_barrier()

    tile.TileContext._drain_and_barrier = _drain_and_barrier


def _split_multi_waits(nc, mybir):
    """walrus here allows only one sem-wait per instruction; hoist extras
    onto same-engine NoOps placed immediately before the instruction."""
    uid = [0]
    for f in nc.m.functions:
        for blk in f.blocks:
            insts = list(blk.instructions)
            new = []
            changed = False
            for inst in insts:
                si = getattr(inst, "sync_info", None)
                if si is not None and len(si.on_wait) > 1:
                    waits = list(si.on_wait)
                    for w in waits[:-1]:
                        uid[0] += 1
                        nop = mybir.InstNoOp(
                            name=f"{inst.name}_wsplit{uid[0]}",
                            sync_info=mybir.SyncInfo(on_wait=[w], on_update=[]),
                            bass_nofuse=True,
                            engine=inst.engine,
                        )
                        new.append(nop)
                    si.on_wait = waits[-1:]
                    changed = True
                new.append(inst)
            if changed:
                blk.instructions = new


def _build(use_bias, use_gb):
    import concourse.bass as bass
    import concourse.tile as tile
    from concourse import mybir
    from concourse.masks import make_identity

    _patch_tile_drain()

    f32 = mybir.dt.float32
    bf16 = mybir.dt.bfloat16
    f16 = mybir.dt.float16
    f8 = mybir.dt.float8e4
    X = mybir.AxisListType.X
    ADD = mybir.AluOpType.add
    MUL = mybir.AluOpType.mult
    SUB = mybir.AluOpType.subtract
    AF = mybir.ActivationFunctionType
    DR = mybir.MatmulPerfMode.DoubleRow

    nc = bass.Bass("TRN2", target_bir_lowering=False, debug=False, num_devices=NCORES)

    xh_d = nc.dram_tensor("xh", [A, TB, PB, KB, PB], f8, kind="ExternalInput")
    xl_d = nc.dram_tensor("xl", [A, TB, PB, KB, PB], f8, kind="ExternalInput")
    xq_d = nc.dram_tensor("xq", [A, TB, PB, KB, PB], f8, kind="ExternalInput")
    x0_d = nc.dram_tensor("x0", [T, H], f16, kind="ExternalInput")
    wq_d = nc.dram_tensor("wq", [H, H], f8, kind="ExternalInput")
    wk_d = nc.dram_tensor("wk", [H, H], f8, kind="ExternalInput")
    wvh_d = nc.dram_tensor("wvh", [H, H], f8, kind="ExternalInput")
    wvl_d = nc.dram_tensor("wvl", [H, H], f8, kind="ExternalInput")
    wo_d = nc.dram_tensor("wo", [H, H], bf16, kind="ExternalInput")
    if use_bias:
        bqkv_d = nc.dram_tensor("bqkv", [3 * H], f32, kind="ExternalInput")
        bo_d = nc.dram_tensor("bo", [H], f32, kind="ExternalInput")
    if use_gb:
        gam_d = nc.dram_tensor("gam", [H], f32, kind="ExternalInput")
        bet_d = nc.dram_tensor("bet", [H], f32, kind="ExternalInput")
    out_d = nc.dram_tensor("out", [T, H], f16, kind="ExternalOutput")



    with tile.TileContext(nc) as tc:
        with (
            tc.tile_pool(name="wpool", bufs=1) as wpool,
            tc.tile_pool(name="xp", bufs=2) as xp,
            tc.tile_pool(name="qk", bufs=2) as qkp,
            tc.tile_pool(name="zp", bufs=2) as zp,
            tc.tile_pool(name="ep", bufs=2) as epool,
            tc.tile_pool(name="att", bufs=2) as att_pool,
            tc.tile_pool(name="vp", bufs=2) as vpool,
            tc.tile_pool(name="res", bufs=2) as res_pool,
            tc.tile_pool(name="mm", bufs=5, space="PSUM") as mm_pool,
            tc.tile_pool(name="sp", bufs=2, space="PSUM") as sp_pool,
            tc.tile_pool(name="tp", bufs=1, space="PSUM") as tp_pool,
        ):
            # --- resident constants -------------------------------------
            identb = wpool.tile([PB, PB], bf16)
            make_identity(nc, identb)
            epst = wpool.tile([PB, 1], f32)
            nc.vector.memset(epst, EPS)
            # per-fb head indicators: col (2fb + (p>=64)) is 1
            inds = []
            for fb in range(FB):
                ind = wpool.tile([PB, NH], bf16, tag=f"ind{fb}")
                nc.vector.memset(ind, 0.0)
                nc.vector.memset(ind[0:64, 2 * fb : 2 * fb + 1], 1.0)
                nc.vector.memset(ind[64:PB, 2 * fb + 1 : 2 * fb + 2], 1.0)
                inds.append(ind)

            # wq first (block-0 qk matmuls need it), then block-0 xh, then
            # the rest of the weights, interleaved so nothing gates the start.
            wq_sb = wpool.tile([PB, KB, H], f8)
            wq_r = wq_d.rearrange("(kb p) n -> p kb n", p=PB)
            nc.sync.dma_start(out=wq_sb[:, :, 0:512], in_=wq_r[:, :, 0:512])
            nc.sync.dma_start(out=wq_sb[:, :, 512:H], in_=wq_r[:, :, 512:H])
            xh0_t = []
            for a in range(A):
                th0 = xp.tile([PB, KB, PB], f8, tag="xh", bufs=8)
                nc.sync.dma_start(out=th0, in_=xh_d[a, 0])
                xh0_t.append(th0)
            wk_sb = wpool.tile([PB, KB, H], f8)
            nc.sync.dma_start(out=wk_sb, in_=wk_d.rearrange("(kb p) n -> p kb n", p=PB))
            xlq0 = []
            for a in range(A):
                tl0 = xp.tile([PB, KB, PB], f8, tag="xl", bufs=8)
                nc.sync.dma_start(out=tl0, in_=xl_d[a, 0])
                tq0 = xp.tile([PB, KB, PB], f8, tag="xq", bufs=8)
                nc.sync.dma_start(out=tq0, in_=xq_d[a, 0])
                xlq0.append((tl0, tq0))
            wvh_sb = wpool.tile([PB, KB, H], f8)
            wvh_r = wvh_d.rearrange("(kb p) n -> p kb n", p=PB)
            nc.sync.dma_start(out=wvh_sb[:, :, 0:512], in_=wvh_r[:, :, 0:512])
            nc.sync.dma_start(out=wvh_sb[:, :, 512:H], in_=wvh_r[:, :, 512:H])
            x00 = xp.tile([PB, H], f16, tag="x0", bufs=4)
            nc.sync.dma_start(out=x00, in_=x0_d[0:PB, :])
            wvl_sb = wpool.tile([PB, KB, H], f8)
            nc.gpsimd.dma_start(out=wvl_sb, in_=wvl_d.rearrange("(kb p) n -> p kb n", p=PB))
            wo_sb = wpool.tile([PB, KB, H], bf16)
            nc.gpsimd.dma_start(out=wo_sb, in_=wo_d.rearrange("(kb p) n -> p kb n", p=PB))

            if use_bias:
                # feature-major qk biases: [128 p, 8 fb] each
                bq_fm = wpool.tile([PB, FB], f32)
                nc.gpsimd.dma_start(out=bq_fm, in_=bqkv_d.ap()[0:H].rearrange("(fb p) -> p fb", p=PB))
                bk_fm = wpool.tile([PB, FB], f32)
                nc.gpsimd.dma_start(out=bk_fm, in_=bqkv_d.ap()[H : 2 * H].rearrange("(fb p) -> p fb", p=PB))
                bv_sb = wpool.tile([PB, H], f32)
                nc.sync.dma_start(out=bv_sb, in_=bqkv_d.ap()[2 * H : 3 * H].unsqueeze(0).broadcast_to([PB, H]))
                bo_sb = wpool.tile([PB, H], f32)
                nc.sync.dma_start(out=bo_sb, in_=bo_d.ap().unsqueeze(0).broadcast_to([PB, H]))
            if use_gb:
                gam_sb = wpool.tile([PB, H], f32)
                nc.sync.dma_start(out=gam_sb, in_=gam_d.ap().unsqueeze(0).broadcast_to([PB, H]))
                bet_sb = wpool.tile([PB, H], f32)
                nc.sync.dma_start(out=bet_sb, in_=bet_d.ap().unsqueeze(0).broadcast_to([PB, H]))

            def load_x(tb_):
                t0_ = tb_ * PB
                xh_t, xl_t, xq_t = [], [], []
                for a in range(A):
                    th = xp.tile([PB, KB, PB], f8, tag="xh", bufs=8)
                    nc.sync.dma_start(out=th, in_=xh_d[a, tb_])
                    xh_t.append(th)
                    tl_ = xp.tile([PB, KB, PB], f8, tag="xl", bufs=8)
                    nc.sync.dma_start(out=tl_, in_=xl_d[a, tb_])
                    xl_t.append(tl_)
                    tq_ = xp.tile([PB, KB, PB], f8, tag="xq", bufs=8)
                    nc.sync.dma_start(out=tq_, in_=xq_d[a, tb_])
                    xq_t.append(tq_)
                x0t = xp.tile([PB, H], f16, tag="x0", bufs=4)
                nc.sync.dma_start(out=x0t, in_=x0_d[t0_ : t0_ + PB, :])
                return xh_t, xl_t, xq_t, x0t

            def o_phase(t0p, accp, x0p):
                # transpose acc (token-major) -> aT for the O projection
                tpo = tp_pool.tile([PB, KB, PB], bf16, tag="tp", bufs=1)
                for kb in range(KB):
                    nc.tensor.transpose(
                        tpo[:, kb, :], accp[:, kb * PB : (kb + 1) * PB], identb
                    )
                aT = res_pool.tile([PB, KB, PB], bf16, tag="aT", bufs=2)
                nc.scalar.copy(out=aT, in_=tpo)

                fused = res_pool.tile([PB, H], bf16, tag="fused", bufs=2)
                for nb in range(2):
                    ops_t = mm_pool.tile([PB, 4, PB], f32, tag="mm")
                    ops = ops_t.rearrange("p a b -> p (a b)")
                    for kb in range(KB):
                        nc.tensor.matmul(
                            ops,
                            lhsT=aT[:, kb, :],
                            rhs=wo_sb[:, kb, nb * 512 : (nb + 1) * 512],
                            start=(kb == 0),
                            stop=(kb == KB - 1),
                        )
                    # residual add
                    nc.vector.tensor_tensor(
                        out=fused[:, nb * 512 : (nb + 1) * 512],
                        in0=ops,
                        in1=x0p[:, nb * 512 : (nb + 1) * 512],
                        op=ADD,
                    )
                if use_bias:
                    nc.vector.tensor_add(out=fused, in0=fused, in1=bo_sb)

                stats = att_pool.tile([PB, 2, 6], f32, tag="stats")
                for g in range(2):
                    nc.vector.bn_stats(
                        out=stats[:, g, :], in_=fused[:, g * 512 : (g + 1) * 512]
                    )
                mv = att_pool.tile([PB, 2], f32, tag="mv")
                nc.vector.bn_aggr(out=mv, in_=stats)
                rstd = att_pool.tile([PB, 1], f32, tag="rstd")
                nc.scalar.activation(
                    out=rstd, in_=mv[:, 1:2], func=AF.Sqrt, bias=epst, scale=1.0
                )
                nc.vector.reciprocal(out=rstd, in_=rstd)
                o16 = res_pool.tile([PB, H], f16, tag="o16", bufs=2)
                if use_gb:
                    tmpf = res_pool.tile([PB, H], f32, tag="tmpf", bufs=2)
                    nc.vector.tensor_scalar(
                        out=tmpf, in0=fused, scalar1=mv[:, 0:1], scalar2=rstd,
                        op0=SUB, op1=MUL,
                    )
                    nc.vector.tensor_mul(out=tmpf, in0=tmpf, in1=gam_sb)
                    nc.vector.tensor_tensor(out=o16, in0=tmpf, in1=bet_sb, op=ADD)
                else:
                    nc.vector.tensor_scalar(
                        out=o16, in0=fused, scalar1=mv[:, 0:1], scalar2=rstd,
                        op0=SUB, op1=MUL,
                    )
                nc.sync.dma_start(out=out_d[t0p : t0p + PB, :], in_=o16)

            def z_phase(q_sb, k_all):
                zts = []
                for i in range(A):
                    zt = zp.tile([PB, FB, A, PB], bf16, tag="z", bufs=6)
                    for j in range(A):
                        nc.vector.tensor_tensor(
                            out=zt[:, :, j, :],
                            in0=q_sb[i],
                            in1=k_all[:, :, j, :],
                            op=MUL,
                        )
                    zts.append(zt)
                return zts

            def score_mm(stm_t, zts, i, j):
                for fb in range(FB):
                    nc.tensor.matmul(
                        stm_t[:, i, j, :],
                        lhsT=zts[i][:, fb, j, :],
                        rhs=inds[fb],
                        start=(fb == 0),
                        stop=(fb == FB - 1),
                    )

            def attn_phase(stm_t, v_sb, t0p, x0p, fast_tail=False):
                    e_sb = epool.tile([PB, A, A, NH], bf16, tag="e", bufs=2)
                    nc.scalar.activation(
                        out=e_sb.rearrange("p i j h -> p (i j h)"),
                        in_=stm_t.rearrange("p i j h -> p (i j h)"),
                        func=AF.Exp,
                        scale=0.125,
                    )
                    e_tm = e_sb

                    # softmax (token-major) -> wbar [128, 16 h, 4 j]
                    rs = att_pool.tile([PB, A, NH], f32, tag="rs")
                    nc.vector.tensor_reduce(
                        out=rs,
                        in_=e_tm.rearrange("p i j h -> p i h j"),
                        axis=X,
                        op=ADD,
                    )
                    rr = att_pool.tile([PB, A, NH], f32, tag="rr")
                    nc.vector.reciprocal(out=rr, in_=rs)
                    wn = att_pool.tile([PB, A, A, NH], bf16, tag="wn")
                    for i in range(A):
                        nc.vector.scalar_tensor_tensor(
                            out=wn[:, i, :, :],
                            in0=e_tm[:, i, :, :],
                            scalar=0.25,
                            in1=rr[:, i, :].unsqueeze(1).broadcast_to([PB, A, NH]),
                            op0=MUL,
                            op1=MUL,
                        )
                    wbar = att_pool.tile([PB, NH, A], f32, tag="wbar")
                    nc.vector.tensor_reduce(
                        out=wbar,
                        in_=wn.rearrange("p i j h -> p h j i"),
                        axis=X,
                        op=ADD,
                    )

                    # expand wbar over head-dim (ACT), weighted v-sum (DVE)
                    acc = res_pool.tile([PB, H], bf16, tag="acc", bufs=2)
                    tmp = res_pool.tile([PB, H], bf16, tag="wvt", bufs=1)
                    for j in range(A):
                        wbe = vpool.tile([PB, NH, HD], bf16, tag="wbe", bufs=2)
                        wbe_eng = nc.scalar if fast_tail else nc.gpsimd
                        if fast_tail:
                            wbe_eng.copy(
                                out=wbe,
                                in_=wbar[:, :, j].unsqueeze(2).broadcast_to([PB, NH, HD]),
                            )
                        else:
                            wbe_eng.tensor_copy(
                                out=wbe,
                                in_=wbar[:, :, j].unsqueeze(2).broadcast_to([PB, NH, HD]),
                            )
                        dst = acc if j == 0 else tmp
                        eng = nc.gpsimd if (j == 3 and not fast_tail) else nc.vector
                        eng.tensor_tensor(
                            out=dst,
                            in0=wbe.rearrange("p h d -> p (h d)"),
                            in1=v_sb[j],
                            op=MUL,
                        )
                        if j > 0:
                            nc.vector.tensor_add(out=acc, in0=acc, in1=tmp)

                    return (t0p, acc, x0p)

            pending_attn = None
            pending_o = None
            xcache = {0: ([t for t in xh0_t], [t for t, _ in xlq0], [t for _, t in xlq0], x00)}
            for tb in range(TB):
                t0 = tb * PB
                xh_t, xl_t, xq_t, x0t = xcache.pop(tb) if tb in xcache else load_x(tb)
                if tb + 1 < TB:
                    xcache[tb + 1] = load_x(tb + 1)

                # --- Q,K feature-major fp8 DR: out [128 f, 128 t] per fb ---
                q_sb = []
                k_all = qkp.tile([PB, FB, A, PB], bf16, tag="k_all", bufs=2)
                for a in range(A):
                    qa = qkp.tile([PB, FB, PB], bf16, tag="q", bufs=5)
                    q_sb.append(qa)
                for proj in range(2):
                    w_sb = wq_sb if proj == 0 else wk_sb
                    for a in range(A):
                        for fbh in range(2):
                            ps = mm_pool.tile([PB, 4, PB], f32, tag="mm")
                            for fbi in range(4):
                                fb = fbh * 4 + fbi
                                col = fb * PB
                                for kp in range(KP):
                                    nc.tensor.matmul(
                                        ps[:, fbi, :],
                                        lhsT=w_sb[:, 2 * kp : 2 * kp + 2, col : col + PB],
                                        rhs=xh_t[a][:, 2 * kp : 2 * kp + 2, :],
                                        start=(kp == 0),
                                        stop=(kp == KP - 1),
                                        perf_mode=DR,
                                    )
                            if proj == 0:
                                dst = q_sb[a][:, fbh * 4 : (fbh + 1) * 4, :]
                            else:
                                dst = k_all[:, fbh * 4 : (fbh + 1) * 4, a, :]
                            if (a + fbh) % 2 == 0:
                                nc.scalar.copy(out=dst, in_=ps)
                            else:
                                nc.vector.tensor_copy(out=dst, in_=ps)
                            if use_bias:
                                bfm = bq_fm if proj == 0 else bk_fm
                                nc.vector.tensor_tensor(
                                    out=dst,
                                    in0=dst,
                                    in1=bfm[:, fbh * 4 : (fbh + 1) * 4]
                                    .unsqueeze(2)
                                    .broadcast_to([PB, 4, PB]),
                                    op=ADD,
                                )

                # --- V token-major 3-term compensated fp8 DR ---------------
                # single PSUM group: xh@Wvh + xl_raw@Wvh + (xh/16)@(Wvl*16)
                # (previous block's tiny score matmuls interleaved to keep the
                # PE sequencer fed with wide matmuls between them)
                stm_prev = None
                if pending_attn is not None:
                    stm_prev = sp_pool.tile([PB, A, A, NH], f32, tag="s")
                    zts_prev = pending_attn[0]
                    score_groups = [(i, j) for i in range(A) for j in range(A)]
                v_sb = []
                for a in range(A):
                    va = vpool.tile([PB, H], bf16, tag="v", bufs=8)
                    for nb in range(2):
                        psa_t = mm_pool.tile([PB, 4, PB], f32, tag="mm")
                        psa = psa_t.rearrange("p a b -> p (a b)")
                        for ti, (xt_, wv_) in enumerate(
                            ((xh_t, wvh_sb), (xl_t, wvh_sb), (xq_t, wvl_sb))
                        ):
                            for kp in range(KP):
                                nc.tensor.matmul(
                                    psa,
                                    lhsT=xt_[a][:, 2 * kp : 2 * kp + 2, :],
                                    rhs=wv_[:, 2 * kp : 2 * kp + 2, nb * 512 : (nb + 1) * 512],
                                    start=(ti == 0 and kp == 0),
                                    stop=(ti == 2 and kp == KP - 1),
                                    perf_mode=DR,
                                )
                        if stm_prev is not None:
                            for _ in range(2):
                                i_, j_ = score_groups.pop(0)
                                score_mm(stm_prev, zts_prev, i_, j_)
                        nc.scalar.copy(out=va[:, nb * 512 : (nb + 1) * 512], in_=psa)
                    if use_bias:
                        nc.vector.tensor_tensor(
                            out=va, in0=va,
                            in1=bv_sb, op=ADD,
                        )
                    v_sb.append(va)

                zts = z_phase(q_sb, k_all)
                if tb == TB - 1 and pending_attn is not None:
                    # last iteration: finish block b-1 attention before o(b-2)
                    new_o = attn_phase(stm_prev, *pending_attn[1:])
                    if pending_o is not None:
                        o_phase(*pending_o)
                    pending_o = new_o
                else:
                    if pending_o is not None:
                        o_phase(*pending_o)
                        pending_o = None
                    if pending_attn is not None:
                        pending_o = attn_phase(stm_prev, *pending_attn[1:])
                pending_attn = (zts, v_sb, t0, x0t)

            if pending_o is not None:
                o_phase(*pending_o)
            stm_last = sp_pool.tile([PB, A, A, NH], f32, tag="s")
            for i_ in range(A):
                for j_ in range(A):
                    score_mm(stm_last, pending_attn[0], i_, j_)
            o_phase(*attn_phase(stm_last, *pending_attn[1:], fast_tail=True))


    _split_multi_waits(nc, mybir)
    return nc


def get_nc(use_bias, use_gb):
    key = (use_bias, use_gb)
    if key not in _BUILD_CACHE:
        _BUILD_CACHE[key] = _build(use_bias, use_gb)
    return _BUILD_CACHE[key]


def make_in_maps(inputs):
    import ml_dtypes

    F8 = ml_dtypes.float8_e4m3
    BF = ml_dtypes.bfloat16

    ao = np.ascontiguousarray(np.asarray(inputs["adapter_outputs"], dtype=np.float32))
    Wq = np.asarray(inputs["Wq"], dtype=np.float32)
    Wk = np.asarray(inputs["Wk"], dtype=np.float32)
    Wv = np.asarray(inputs["Wv"], dtype=np.float32)
    Wo = np.asarray(inputs["Wo"], dtype=np.float32)
    bqkv = np.concatenate(
        [np.asarray(inputs["bq"]), np.asarray(inputs["bk"]), np.asarray(inputs["bv"])]
    ).astype(np.float32)
    bo = np.asarray(inputs["bo"], dtype=np.float32)
    gam = np.asarray(inputs["ln_gamma"], dtype=np.float32)
    bet = np.asarray(inputs["ln_beta"], dtype=np.float32)

    use_bias = bool(np.any(bqkv != 0.0) or np.any(bo != 0.0))
    use_gb = bool(np.any(gam != 1.0) or np.any(bet != 0.0))

    wq8 = np.ascontiguousarray(Wq).astype(F8)
    wk8 = np.ascontiguousarray(Wk).astype(F8)
    wvh8 = Wv.astype(F8)
    wvl8 = ((Wv - wvh8.astype(np.float32)) * 16.0).astype(F8)
    wo16 = Wo.astype(BF)

    in_maps = []
    for c in range(NCORES):
        row0 = c * T
        b = row0 // S
        s0 = row0 % S
        xc = ao[:, b, s0 : s0 + T, :]  # [A, T, H]
        xt = np.ascontiguousarray(xc.transpose(0, 2, 1))  # [A, H, T]
        xh8 = xt.astype(F8)
        xh32 = xh8.astype(np.float32)
        xl8 = (xt - xh32).astype(F8)
        xq8 = (xh32 / 16.0).astype(F8)

        def repack(arr):  # [A, H, T] -> [A, TB, PB(p), KB, PB(t)] contiguous
            return np.ascontiguousarray(
                arr.reshape(A, KB, PB, TB, PB).transpose(0, 3, 2, 1, 4)
            )

        m = {
            "xh": repack(xh8),
            "xl": repack(xl8),
            "xq": repack(xq8),
            "x0": np.ascontiguousarray(xc[0]).astype(np.float16),
            "wq": wq8,
            "wk": wk8,
            "wvh": wvh8,
            "wvl": wvl8,
            "wo": wo16,
        }
        if use_bias:
            m["bqkv"] = bqkv
            m["bo"] = bo
        if use_gb:
            m["gam"] = gam
            m["bet"] = bet
        in_maps.append(m)
    return in_maps, use_bias, use_gb


def assemble(results):
    out = np.empty((B, S, H), dtype=np.float32)
    for c in range(NCORES):
        row0 = c * T
        b = row0 // S
        s0 = row0 % S
        out[b, s0 : s0 + T, :] = np.asarray(results[c]["out"]).astype(np.float32)
    return out


def kernel(**inputs):
    from concourse.bass_utils import run_bass_kernel_spmd

    in_maps, use_bias, use_gb = make_in_maps(inputs)
    nc = get_nc(use_bias, use_gb)
    res = run_bass_kernel_spmd(nc, in_maps, list(range(NCORES)))
    return assemble(res.results)
